# revision 31
# baseline (speedup 1.0000x reference)
"""Trainium2 Bass kernel for nn_AttentionContextEncoder (gnn_message_passing).

reference:
  ents = ctx.T.reshape(B, 7, 4)
  prop_emb = relu(ents @ w_prop + b_prop)                      # [B,7,128]
  diffs[b,i,j,:] = ents[b,i,:] - ents[b,j,:]
  dist = sqrt(diffs[...,0]^2 + diffs[...,1]^2)
  rel = relu(concat([diffs, dist]) @ w_rel + b_rel)            # [B,7,7,128]
  rel_emb = sum_{j != i} rel[:, i, j, :]                       # [B,7,128]
  out = concat([prop_emb, rel_emb], -1)                        # [B,7,256]

v2 design (data-parallel over 8 cores, B=2048/core):
- Host prebuilds constants: gather/difference matrix G (PE computes all 21
  pair diffs + 7 prop blocks in one matmul group), squared-distance
  reduction matrix R, and three weight images with the bias folded in as an
  extra contraction row against a ones-row in the rhs (no bias APs needed).
- Prep: G-matmul -> diffs/props in PSUM -> ACT copy to bf16, DVE square,
  R-matmul -> dist^2 -> ACT sqrt -> dist; ~35 small DMAs stage the strip
  layout rhs6/prop6.
- Main: per directed pair one 6-row matmul per [128,1024] PSUM slot (x2
  halves); drains split between ACT (relu -> bf16 r-tiles, summed on
  GpSimd) and DVE (fused relu+accumulate scalar_tensor_tensor chains).
- Output bf16 [2,7,128,B] per core; host concatenates, converts to f32,
  and transposes.
"""
import numpy as np
import ml_dtypes
from contextlib import ExitStack

import concourse.bass as bass
import concourse.bacc as bacc
import concourse.mybir as mybir
import concourse.tile as tile
from concourse.bass_utils import run_bass_kernel_spmd

F32 = mybir.dt.float32
BF16 = mybir.dt.bfloat16
AF = mybir.ActivationFunctionType
ALU = mybir.AluOpType

NUM_ENT = 7
DIM_ENT = 4
H = 128
B_TOTAL = 16384
N_CORES = 8
B = B_TOTAL // N_CORES          # 2048 per core
HB = B // 2                     # 1024 half-batch (one PSUM slot)

# ---- pair table: K7 edge-coloring so each target's 6 pairs spread over
# 4 strips (<=2 per strip) -> consecutive matmuls hit different PE row
# groups and run concurrently. class c = {(i,j): i+j = c mod 7}; strips
# take classes {0,1},{2,3},{4,5},{6} -> sizes {6,6,6,3}.
_CLS = [[] for _ in range(7)]
for i in range(NUM_ENT):
    for j in range(i + 1, NUM_ENT):
        _CLS[(i + j) % 7].append((i, j))
_STRIP_PAIRS = [_CLS[0] + _CLS[1], _CLS[2] + _CLS[3], _CLS[4] + _CLS[5], _CLS[6]]
PAIRS = [p for sp in _STRIP_PAIRS for p in sp]       # ordinal = strip-major
STRIP_NP = [len(sp) for sp in _STRIP_PAIRS]          # [6, 6, 6, 3]
STRIP_START = [0, 6, 12, 18]
PAIR_SG = {}
for s in range(4):
    for g in range(STRIP_NP[s]):
        PAIR_SG[STRIP_START[s] + g] = (s, g)
PAIR_IDX = {PAIRS[k]: k for k in range(21)}

# prop t -> (strip, page) in prop6
PROP_SG = {t: (t % 4, t // 4) for t in range(NUM_ENT)}

# prop (t) halves drained on DVE instead of ACT to balance engines
PROP_DVE = {5, 6}
# targets whose final acc+=c1 runs on GpSimd (others on DVE)
FINAL_GPS = {0, 1, 2, 3}


def _ordered_pairs(t):
    """t's partners ordered round-robin over strips for PE concurrency."""
    by_strip = [[] for _ in range(4)]
    for j in range(NUM_ENT):
        if j == t:
            continue
        a, b_ = (t, j) if t < j else (j, t)
        s, _ = PAIR_SG[PAIR_IDX[(a, b_)]]
        by_strip[s].append(j)
    order = []
    r = 0
    while len(order) < 6:
        for s in range(4):
            if len(by_strip[s]) > r:
                order.append(by_strip[s][r])
        r += 1
    return order


def build_constants(w_prop, b_prop, w_rel, b_rel):
    """Host-side constant arrays (bf16)."""
    bf = ml_dtypes.bfloat16
    # G: [28, 112]: col 21c+k = diff dim c of pair k; col 84+4t+c = ctx row 4t+c
    G = np.zeros((NUM_ENT * DIM_ENT, 112), np.float32)
    for k, (i, j) in enumerate(PAIRS):
        for c in range(DIM_ENT):
            G[4 * i + c, 21 * c + k] = 1.0
            G[4 * j + c, 21 * c + k] = -1.0
    for t in range(NUM_ENT):
        for c in range(DIM_ENT):
            G[4 * t + c, 84 + 4 * t + c] = 1.0
    # R: [112, 21]: col k sums sq rows k (dd0^2) and 21+k (dd1^2)
    R = np.zeros((112, 21), np.float32)
    for k in range(21):
        R[k, k] = 1.0
        R[21 + k, k] = 1.0
    # weight images [128, 128]: per strip s rows 32s+r:
    #   r=0..3: +/-w_rel[0:4]; r=4: w_rel[4]; r=5: b_rel   (wp / wm)
    #   wq: r=0..3: w_prop; r=4: b_prop
    wp = np.zeros((H, H), np.float32)
    wm = np.zeros((H, H), np.float32)
    wq = np.zeros((H, H), np.float32)
    for s in range(4):
        r0 = 32 * s
        wp[r0:r0 + 4, :] = w_rel[0:4]
        wp[r0 + 4, :] = w_rel[4]
        wp[r0 + 5, :] = b_rel
        wm[r0:r0 + 4, :] = -w_rel[0:4]
        wm[r0 + 4, :] = w_rel[4]
        wm[r0 + 5, :] = b_rel
        wq[r0:r0 + 4, :] = w_prop
        wq[r0 + 4, :] = b_prop
    return {
        "gmat": G.astype(bf), "rmat": R.astype(bf),
        "wpimg": wp.astype(bf), "wmimg": wm.astype(bf), "wqimg": wq.astype(bf),
    }


def build():
    nc = bacc.Bacc("TRN2", target_bir_lowering=False, debug=False,
                   num_devices=N_CORES)
    ctxb_d = nc.dram_tensor("ctxb", [NUM_ENT * DIM_ENT, B], BF16,
                            kind="ExternalInput").ap()
    gmat_d = nc.dram_tensor("gmat", [NUM_ENT * DIM_ENT, 112], BF16,
                            kind="ExternalInput").ap()
    rmat_d = nc.dram_tensor("rmat", [112, 21], BF16, kind="ExternalInput").ap()
    wp_d = nc.dram_tensor("wpimg", [H, H], BF16, kind="ExternalInput").ap()
    wm_d = nc.dram_tensor("wmimg", [H, H], BF16, kind="ExternalInput").ap()
    wq_d = nc.dram_tensor("wqimg", [H, H], BF16, kind="ExternalInput").ap()
    out_d = nc.dram_tensor("out", [2, NUM_ENT, H, B], BF16,
                           kind="ExternalOutput").ap()

    with tile.TileContext(nc) as tc, ExitStack() as ctx:
        stat = ctx.enter_context(tc.tile_pool(name="stat", bufs=1))
        rp = ctx.enter_context(tc.tile_pool(name="rp", bufs=10))
        accp = ctx.enter_context(tc.tile_pool(name="accp", bufs=3))
        poutp = ctx.enter_context(tc.tile_pool(name="poutp", bufs=3))
        cbp = ctx.enter_context(tc.tile_pool(name="cbp", bufs=3))

        psl = ctx.enter_context(tc.tile_pool(name="psl", bufs=4, space="PSUM"))

        # ---------- inputs ----------
        ctxb = stat.tile([NUM_ENT * DIM_ENT, B], BF16)
        nc.sync.dma_start(ctxb[:], ctxb_d[:])
        gm = stat.tile([NUM_ENT * DIM_ENT, 112], BF16)
        nc.sync.dma_start(gm[:], gmat_d[:])
        rm = stat.tile([112, 21], BF16)
        nc.sync.dma_start(rm[:], rmat_d[:])
        wpimg = stat.tile([H, H], BF16)
        nc.sync.dma_start(wpimg[:], wp_d[:])
        wmimg = stat.tile([H, H], BF16)
        nc.sync.dma_start(wmimg[:], wm_d[:])
        wqimg = stat.tile([H, H], BF16)
        nc.sync.dma_start(wqimg[:], wq_d[:])

        ones = stat.tile([6, B], BF16)
        nc.vector.memset(ones[:], 1.0)

        # ---------- prep: G-matmul -> cmp -> cmpb, sq, R-matmul -> dist ----
        cmpb = stat.tile([112, B], BF16)
        for h in range(2):
            cslot = psl.tile([112, HB], F32, tag="slot", name="cslot")
            for c in range(2):
                nc.tensor.matmul(cslot[:, 512 * c:512 * c + 512],
                                 gm[:, :],
                                 ctxb[:, HB * h + 512 * c:HB * h + 512 * c + 512],
                                 start=True, stop=True, tile_position=(0, 0))
            nc.scalar.copy(cmpb[:, HB * h:HB * h + HB], cslot[:])
        sq = stat.tile([42, B], BF16)
        nc.vector.tensor_mul(sq[0:42, :], cmpb[0:42, :], cmpb[0:42, :])
        distb = stat.tile([21, B], BF16)
        for h in range(2):
            dslot = psl.tile([21, HB], F32, tag="slot", name="dslot")
            for c in range(2):
                nc.tensor.matmul(dslot[:, 512 * c:512 * c + 512],
                                 rm[0:42, :],
                                 sq[0:42, HB * h + 512 * c:HB * h + 512 * c + 512],
                                 start=True, stop=True, tile_position=(0, 0))
            nc.scalar.activation(distb[:, HB * h:HB * h + HB], dslot[:], AF.Sqrt)

        # ---------- staging (spread across idle engine queues) ----------
        # rhs6 [128, 6, B]: strip s rows 32s+0..3 diffs, +4 dist, +5 ones
        rhs6 = stat.tile([H, 6, B], BF16)
        prop6 = stat.tile([H, 2, B], BF16)
        for s in range(4):
            k0, np_ = STRIP_START[s], STRIP_NP[s]
            eng0 = nc.gpsimd if s < 2 else nc.sync
            for c in range(DIM_ENT):
                eng0.dma_start(rhs6[32 * s + c:32 * s + c + 1, 0:np_, :],
                               cmpb[21 * c + k0:21 * c + k0 + np_, :])
            nc.sync.dma_start(rhs6[32 * s + 4:32 * s + 5, 0:np_, :],
                              distb[k0:k0 + np_, :])
            nc.gpsimd.dma_start(rhs6[32 * s + 5:32 * s + 6, 0:6, :],
                                ones[0:6, :])
            nc.gpsimd.dma_start(prop6[32 * s + 4:32 * s + 5, 0:2, :],
                                ones[0:2, :])
        for t in range(NUM_ENT):
            ps_, pg_ = PROP_SG[t]
            nc.scalar.dma_start(prop6[32 * ps_:32 * ps_ + 4, pg_, :],
                                cmpb[84 + 4 * t:84 + 4 * t + 4, :])

        # ---------- main ----------
        def rel_mm(t, j, h):
            """matmuls for directed pair (t -> j), batch half h -> psum slot"""
            a, b_ = (t, j) if t < j else (j, t)
            s, g = PAIR_SG[PAIR_IDX[(a, b_)]]
            img = wpimg if t < j else wmimg
            slot = psl.tile([H, HB], F32, tag="slot", name="gslot")
            for c in range(2):
                nc.tensor.matmul(
                    slot[:, 512 * c:512 * c + 512],
                    img[32 * s:32 * s + 6, :],
                    rhs6[32 * s:32 * s + 6, g,
                         HB * h + 512 * c:HB * h + 512 * c + 512],
                    start=True, stop=True, tile_position=(32 * s, 0))
            return slot

        def prop_mm(t, h):
            ps_, pg_ = PROP_SG[t]
            slot = psl.tile([H, HB], F32, tag="slot", name="pslot")
            for c in range(2):
                nc.tensor.matmul(
                    slot[:, 512 * c:512 * c + 512],
                    wqimg[32 * ps_:32 * ps_ + 5, :],
                    prop6[32 * ps_:32 * ps_ + 5, pg_,
                          HB * h + 512 * c:HB * h + 512 * c + 512],
                    start=True, stop=True, tile_position=(32 * ps_, 0))
            return slot

        for t in range(NUM_ENT):
            order = _ordered_pairs(t)
            # alternate roles so ACT and DVE drain concurrently; consecutive
            # slots also alternate strips (order is strip-round-robin)
            a_set = [order[0], order[2], order[4]]
            d_set = [order[1], order[3], order[5]]
            acc = accp.tile([H, B], BF16, tag="acc", name="acc")
            rbufs = [[None, None] for _ in range(3)]
            c1 = [None, None]
            for step in range(3):
                aj, dj = a_set[step], d_set[step]
                for h in range(2):
                    slot = rel_mm(t, aj, h)
                    r = rp.tile([H, HB], BF16, tag="r", name="r")
                    nc.scalar.activation(r[:], slot[:], AF.Relu)
                    rbufs[step][h] = r
                for h in range(2):
                    slot = rel_mm(t, dj, h)
                    in1 = (rbufs[0][h][:] if step == 0
                           else acc[:, HB * h:HB * h + HB])
                    nc.vector.scalar_tensor_tensor(
                        acc[:, HB * h:HB * h + HB], slot[:], 0.0, in1,
                        op0=ALU.max, op1=ALU.add)
                if step == 2:
                    for h in range(2):
                        c = cbp.tile([H, HB], BF16, tag="c1", name="c1")
                        nc.gpsimd.tensor_add(c[:], rbufs[1][h][:],
                                             rbufs[2][h][:])
                        c1[h] = c
            # final acc += c1
            for h in range(2):
                eng = nc.gpsimd if t in FINAL_GPS else nc.vector
                eng.tensor_add(acc[:, HB * h:HB * h + HB],
                               acc[:, HB * h:HB * h + HB], c1[h][:])
            nc.sync.dma_start(out_d[1, t, :, :], acc[:])

            # prop
            pout = poutp.tile([H, B], BF16, tag="pout", name="pout")
            for h in range(2):
                slot = prop_mm(t, h)
                if t in PROP_DVE:
                    nc.vector.tensor_single_scalar(
                        pout[:, HB * h:HB * h + HB], slot[:], 0.0, op=ALU.max)
                else:
                    nc.scalar.activation(pout[:, HB * h:HB * h + HB],
                                         slot[:], AF.Relu)
            nc.sync.dma_start(out_d[0, t, :, :], pout[:])

    nc.compile()
    return nc


_NC_CACHE = None


def _get_nc():
    global _NC_CACHE
    if _NC_CACHE is None:
        _NC_CACHE = build()
    return _NC_CACHE


def run(ctx, w_prop, b_prop, w_rel, b_rel, trace=False):
    bf = ml_dtypes.bfloat16
    ctx = np.asarray(ctx, dtype=np.float32)
    nc = _get_nc()
    shared = build_constants(np.asarray(w_prop, np.float32),
                             np.asarray(b_prop, np.float32),
                             np.asarray(w_rel, np.float32),
                             np.asarray(b_rel, np.float32))
    in_maps = []
    for c in range(N_CORES):
        m = dict(shared)
        m["ctxb"] = np.ascontiguousarray(ctx[:, c * B:(c + 1) * B]).astype(bf)
        in_maps.append(m)
    res = run_bass_kernel_spmd(nc, in_maps, core_ids=list(range(N_CORES)),
                               trace=trace)
    shards = [np.asarray(res.results[c]["out"]).astype(np.float32)
              for c in range(N_CORES)]
    full = np.concatenate(shards, axis=3)                     # [2,7,128,16384]
    out = np.transpose(full, (3, 1, 0, 2)).reshape(B_TOTAL, NUM_ENT, 2 * H)
    return np.ascontiguousarray(out), res


def kernel(ctx, w_prop, b_prop, w_rel, b_rel):
    return run(ctx, w_prop, b_prop, w_rel, b_rel)[0]
